# revision 2
# baseline (speedup 1.0000x reference)
"""GCNConv Trainium2 kernel: out = segment_sum(w_e * (x @ W)[src_e] -> dst_e) + bias.

Distribution (8-core SPMD, one program):
  - Destination nodes are bin-packed (LPT over per-dst edge counts) into
    8*98 = 784 windows of <=128 dsts each, so every (core, window) has an
    almost equal edge count; windows pad to a uniform 16 blocks of 128 edges.
  - Aggregation runs in x-space (in_dim features): per 128-edge block one PE
    matmul aggT += Xg^T @ S accumulates into the window's PSUM tile; at window
    end aggT moves to SBUF and out = aggT^T @ W + bias is stored.

Per core:
  - 98 windows are split into 7 groups of 14. For each (core, group) the
    host stores the deduplicated x rows used by that group's edges in a
    per-core DRAM pool (~25k rows < int16 gather reach), so each gather is
    one big 4096-row dma_gather at a single base.
  - S ([128 edges, 128 dst] scaled one-hot) is built on-device: one DVE
    tensor_scalar (iota == dstoff) * w per block, fed by tiny f32 metadata
    columns -- no S streaming from DRAM.
"""

import sys

sys.path.insert(0, "/opt/trn_rl_repo")

import heapq

import ml_dtypes
import numpy as np

from concourse import bacc, bass, mybir, tile
from concourse.bass_utils import run_bass_kernel_spmd

N_CORES = 8
P = 128  # partitions / block size / dst window size
NWIN = 98  # windows per core
GW = 14  # windows per dedup group
NG = NWIN // GW  # 7 groups
GCH = 4096  # gather chunk: slots per dma_gather instruction


def _preprocess(n_nodes, edge_index, edge_weight):
    """Bin-pack dsts, build per-core tapes + dedup row pools."""
    nbins = N_CORES * NWIN
    dst = edge_index[0].astype(np.int64)
    src = edge_index[1].astype(np.int64)
    w = edge_weight.astype(np.float32)
    E = dst.shape[0]

    # --- LPT bin-packing of dsts into 784 windows (cap 128 dsts each) ---
    cnt_dst = np.bincount(dst, minlength=n_nodes)
    order = np.argsort(-cnt_dst, kind="stable")
    heap = [(0, b) for b in range(nbins)]  # (sum, bin)
    heapq.heapify(heap)
    bin_of_dst = np.empty(n_nodes, np.int64)
    off_of_dst = np.empty(n_nodes, np.int64)
    bin_fill = np.zeros(nbins, np.int64)
    stash = []
    for d in order:
        while True:
            s, b = heapq.heappop(heap)
            if bin_fill[b] < P:
                break
            stash.append(None)  # full bin: drop from heap
        bin_of_dst[d] = b
        off_of_dst[d] = bin_fill[b]
        bin_fill[b] += 1
        heapq.heappush(heap, (s + cnt_dst[d], b))
    core_of_bin = np.arange(nbins) // NWIN
    win_of_bin = np.arange(nbins) % NWIN

    core = core_of_bin[bin_of_dst[dst]]
    win = win_of_bin[bin_of_dst[dst]]
    off = off_of_dst[dst].astype(np.float32)

    # uniform blocks per window
    wcnt = np.bincount(bin_of_dst[dst], minlength=nbins)
    BW = int(-(-wcnt.max() // P))
    WSL = BW * P  # slots per window
    B = NWIN * BW  # blocks per core
    SL = B * P  # slots per core

    # --- per-(core,group) dedup of srcs; local idx for each edge ---
    group = win // GW
    cg = core * NG + group  # 0..55
    key = cg * n_nodes + src
    uniq, inv = np.unique(key, return_inverse=True)
    seg_of_uniq = uniq // n_nodes
    seg_sizes = np.bincount(seg_of_uniq, minlength=N_CORES * NG)
    U_max = int(seg_sizes.max())
    assert U_max <= 32767, f"group dedup overflow: {U_max}"
    seg_start = np.concatenate([[0], np.cumsum(seg_sizes)])[:-1]
    idx_local = (inv - seg_start[cg]).astype(np.int16)

    # --- slot position of each edge: window-major, arrival order ---
    wkey = (core * NWIN + win).astype(np.int64)
    order_e = np.argsort(wkey, kind="stable")
    swkey = wkey[order_e]
    starts = np.r_[0, np.flatnonzero(np.diff(swkey)) + 1]
    run_len = np.diff(np.r_[starts, E])
    run_id = np.repeat(np.arange(len(starts)), run_len)
    pos_in_win = np.arange(E) - starts[run_id]
    slot = (swkey % NWIN) * WSL + pos_in_win  # per-core slot

    idx_arr = np.zeros((N_CORES, SL), np.int16)
    off_arr = np.zeros((N_CORES, SL), np.float32)
    w_arr = np.zeros((N_CORES, SL), np.float32)
    flat = (swkey // NWIN) * SL + slot
    idx_arr.reshape(-1)[flat] = idx_local[order_e]
    off_arr.reshape(-1)[flat] = off[order_e]
    w_arr.reshape(-1)[flat] = w[order_e]

    # idx tape wrapped in 16 partitions, replicated 8x: idx[16g+p, s] = tape[16s+p]
    idxw = idx_arr.reshape(N_CORES, SL // 16, 16).transpose(0, 2, 1)
    idx_np = np.tile(idxw, (1, 8, 1)).copy()  # [C, 128, SL//16]

    # metadata columns: [128, B] f32, column b = slots [b*128, (b+1)*128)
    dstf = off_arr.reshape(N_CORES, B, P).transpose(0, 2, 1).copy()
    wf = w_arr.reshape(N_CORES, B, P).transpose(0, 2, 1).copy()

    # per-core dedup row pools (filled later with x data)
    rows_of_uniq = uniq % n_nodes
    return dict(
        idx=idx_np,
        dstf=dstf,
        wf=wf,
        B=B,
        BW=BW,
        U_max=U_max,
        seg_sizes=seg_sizes,
        seg_start=seg_start,
        rows_of_uniq=rows_of_uniq,
        bin_of_dst=bin_of_dst,
        off_of_dst=off_of_dst,
    )


def _build_xg(x_bf, pp):
    """Per-core [NG*U_max, in_dim] bf16 dedup row pools."""
    n, in_dim = x_bf.shape
    U_max = pp["U_max"]
    xg = np.zeros((N_CORES, NG * U_max, in_dim), ml_dtypes.bfloat16)
    for c in range(N_CORES):
        for g in range(NG):
            s = c * NG + g
            rows = pp["rows_of_uniq"][
                pp["seg_start"][s] : pp["seg_start"][s] + pp["seg_sizes"][s]
            ]
            xg[c, g * U_max : g * U_max + len(rows)] = x_bf[rows]
    return xg


def _build_program(in_dim, out_dim, pp):
    B, BW, U_max = pp["B"], pp["BW"], pp["U_max"]
    SL = B * P

    nc = bacc.Bacc(
        "TRN2",
        target_bir_lowering=False,
        debug=False,
        num_devices=N_CORES,
        num_swdge_queues=4,
        dynamic_dma_scratch_size=98304,
    )
    f32 = mybir.dt.float32
    bf16 = mybir.dt.bfloat16
    i16 = mybir.dt.int16

    xg_d = nc.declare_dram_parameter("xg", [NG * U_max, in_dim], bf16, isOutput=False)
    idx_d = nc.declare_dram_parameter("idx", [P, SL // 16], i16, isOutput=False)
    dstf_d = nc.declare_dram_parameter("dstf", [P, B], f32, isOutput=False)
    wf_d = nc.declare_dram_parameter("wf", [P, B], f32, isOutput=False)
    iota_d = nc.declare_dram_parameter("iotab", [P, P], bf16, isOutput=False)
    wmat_d = nc.declare_dram_parameter("wmat", [in_dim, out_dim], f32, isOutput=False)
    bias_d = nc.declare_dram_parameter("biasrep", [P, out_dim], f32, isOutput=False)
    out_d = nc.declare_dram_parameter("out", [NWIN * P, out_dim], f32, isOutput=True)

    eq, mu = mybir.AluOpType.is_equal, mybir.AluOpType.mult
    # gather chunks: per group, GW*BW blocks chopped into GCH-slot chunks
    CB = GCH // P  # blocks per chunk
    chunks = []  # (group, block_start, n_blocks)
    for g in range(NG):
        b0 = g * GW * BW
        bend = (g + 1) * GW * BW
        while b0 < bend:
            nb = min(CB, bend - b0)
            chunks.append((g, b0, nb))
            b0 += nb

    with tile.TileContext(nc) as tc:
        with (
            tc.tile_pool(name="const", bufs=1) as const_tp,
            tc.tile_pool(name="meta", bufs=1) as meta_tp,
            tc.tile_pool(name="g", bufs=5) as g_tp,
            tc.tile_pool(name="s", bufs=10) as s_tp,
            tc.tile_pool(name="aggsb", bufs=3) as agg_tp,
            tc.tile_pool(name="outsb", bufs=3) as outsb_tp,
            tc.tile_pool(name="psum_agg", bufs=6, space="PSUM") as psum_agg_tp,
            tc.tile_pool(name="psum_out", bufs=2, space="PSUM") as psum_out_tp,
        ):
            wmat_t = const_tp.tile([in_dim, out_dim], f32)
            nc.sync.dma_start(out=wmat_t[:], in_=wmat_d[:, :])
            bias_t = const_tp.tile([P, out_dim], f32)
            nc.sync.dma_start(out=bias_t[:], in_=bias_d[:, :])
            iota_t = const_tp.tile([P, P], bf16)
            nc.sync.dma_start(out=iota_t[:], in_=iota_d[:, :])

            idx_t = meta_tp.tile([P, SL // 16], i16)
            nc.sync.dma_start(out=idx_t[:], in_=idx_d[:, :])
            dstf_t = meta_tp.tile([P, B], f32)
            nc.sync.dma_start(out=dstf_t[:], in_=dstf_d[:, :])
            wf_t = meta_tp.tile([P, B], f32)
            nc.sync.dma_start(out=wf_t[:], in_=wf_d[:, :])

            # chunk tiles created lazily in block order
            g_tiles = {}

            def ensure_gather(ci):
                if ci in g_tiles:
                    return
                g, b0, nb = chunks[ci]
                g_t = g_tp.tile([P, nb * in_dim], bf16, tag="g")
                nc.gpsimd.dma_gather(
                    out_ap=g_t[:].rearrange("p (c e) -> p c e", e=in_dim),
                    in_ap=xg_d[g * U_max :, :],
                    idxs_ap=idx_t[:, b0 * P // 16 : (b0 + nb) * P // 16],
                    num_idxs=nb * P,
                    num_idxs_reg=nb * P,
                    elem_size=in_dim,
                    single_packet=False,
                    queue_num=ci % 4,
                )
                g_tiles[ci] = (g_t, b0)

            for w in range(NWIN):
                psum_w = psum_agg_tp.tile([in_dim, P], f32, tag="aggT")
                for j in range(BW):
                    b = w * BW + j
                    ci = b // CB
                    ensure_gather(ci)
                    # prefetch next chunk early
                    if ci + 1 < len(chunks) and b % CB >= CB - 4:
                        ensure_gather(ci + 1)
                    g_t, b0 = g_tiles[ci]
                    s_t = s_tp.tile([P, P], bf16, tag="s")
                    nc.vector.tensor_scalar(
                        out=s_t[:],
                        in0=iota_t[:],
                        scalar1=dstf_t[:, b : b + 1],
                        scalar2=wf_t[:, b : b + 1],
                        op0=eq,
                        op1=mu,
                    )
                    rel = b - b0
                    nc.tensor.matmul(
                        out=psum_w[:],
                        lhsT=g_t[:, rel * in_dim : (rel + 1) * in_dim],
                        rhs=s_t[:],
                        start=(j == 0),
                        stop=(j == BW - 1),
                    )
                agg_sb = agg_tp.tile([in_dim, P], f32, tag="aggsb")
                nc.scalar.copy(out=agg_sb[:], in_=psum_w[:])
                out_ps = psum_out_tp.tile([P, out_dim], f32, tag="outps")
                nc.tensor.matmul(
                    out=out_ps[:], lhsT=agg_sb[:], rhs=wmat_t[:], start=True, stop=True
                )
                out_sb = outsb_tp.tile([P, out_dim], f32, tag="outsb")
                nc.vector.tensor_add(out=out_sb[:], in0=out_ps[:], in1=bias_t[:])
                nc.scalar.dma_start(out=out_d[w * P : (w + 1) * P, :], in_=out_sb[:])

    nc.compile()
    return nc


def _prepare(x, edge_index, edge_weight, weight, bias):
    x = np.asarray(x, np.float32)
    edge_index = np.asarray(edge_index, np.int32)
    edge_weight = np.asarray(edge_weight, np.float32)
    weight = np.asarray(weight, np.float32)
    bias = np.asarray(bias, np.float32)

    n_nodes, in_dim = x.shape
    out_dim = weight.shape[1]

    pp = _preprocess(n_nodes, edge_index, edge_weight)
    nc = _build_program(in_dim, out_dim, pp)

    xg = _build_xg(x.astype(ml_dtypes.bfloat16), pp)
    iotab = np.broadcast_to(
        np.arange(P, dtype=np.float32), (P, P)
    ).astype(ml_dtypes.bfloat16)
    biasrep = np.broadcast_to(bias, (P, out_dim)).astype(np.float32).copy()
    in_maps = [
        {
            "xg": xg[c],
            "idx": pp["idx"][c],
            "dstf": pp["dstf"][c],
            "wf": pp["wf"][c],
            "iotab": iotab.copy(),
            "wmat": weight,
            "biasrep": biasrep,
        }
        for c in range(N_CORES)
    ]
    return nc, in_maps, pp, n_nodes, out_dim


def _collect(res, pp, n_nodes, out_dim):
    out = np.zeros((n_nodes, out_dim), np.float32)
    bin_of_dst, off_of_dst = pp["bin_of_dst"], pp["off_of_dst"]
    dsts = np.arange(n_nodes)
    c = bin_of_dst // NWIN
    row = (bin_of_dst % NWIN) * P + off_of_dst
    for ci in range(N_CORES):
        m = c == ci
        out[dsts[m]] = res.results[ci]["out"][row[m]]
    return out


def kernel(x, edge_index, edge_weight, weight, bias):
    nc, in_maps, pp, n_nodes, out_dim = _prepare(
        x, edge_index, edge_weight, weight, bias
    )
    res = run_bass_kernel_spmd(nc, in_maps, core_ids=list(range(N_CORES)))
    return _collect(res, pp, n_nodes, out_dim)


if __name__ == "__main__":
    rng = np.random.default_rng(0)
    N, E, DI, DO = 100000, 1600000, 128, 64
    if len(sys.argv) > 1 and sys.argv[1] == "small":
        N, E = 20000, 320000
    x = rng.standard_normal((N, DI), dtype=np.float32)
    ei = rng.integers(0, N, (2, E)).astype(np.int32)
    ew = rng.random(E, dtype=np.float32)
    wm = rng.standard_normal((DI, DO), dtype=np.float32) * 0.125
    bs = rng.standard_normal(DO, dtype=np.float32)

    out = kernel(x, ei, ew, wm, bs)

    h = x @ wm
    ref = np.zeros((N, DO), np.float32)
    np.add.at(ref, ei[0], ew[:, None] * h[ei[1]])
    ref += bs
    err = np.abs(out - ref).max() / (np.abs(ref).max() + 1e-9)
    print("max rel err:", err)


# revision 6
# speedup vs baseline: 2.1651x; 2.1651x over previous
"""GCNConv Trainium2 kernel: out = segment_sum(w_e * (x @ W)[src_e] -> dst_e) + bias.

Distribution (8-core SPMD, one program):
  - Destination nodes are bin-packed (LPT over per-dst edge counts) into
    8*98 = 784 windows of <=128 dsts each, so every (core, window) has an
    almost equal edge count; windows pad to a uniform 16 blocks of 128 edges.
  - Aggregation runs in x-space (in_dim features): per 128-edge block one PE
    matmul aggT += Xg^T @ S accumulates into the window's PSUM tile; at window
    end aggT moves to SBUF and out = aggT^T @ W + bias is stored.

Per core:
  - 98 windows are split into 7 groups of 14. For each (core, group) the
    host stores the deduplicated x rows used by that group's edges in a
    per-core DRAM pool (~25k rows < int16 gather reach), so each gather is
    one big 4096-row dma_gather at a single base.
  - S ([128 edges, 128 dst] scaled one-hot) is built on-device per 4096-slot
    gather chunk with two wide DVE tensor_tensor ops on stride-0 broadcast
    views: t1 = (iota == dstoff_bcast); S = t1 * w_bcast -- no S streaming
    from DRAM, no per-block scalar-pointer ops.
"""

import sys

sys.path.insert(0, "/opt/trn_rl_repo")

import heapq

import ml_dtypes
import numpy as np

from concourse import bacc, bass, mybir, tile
from concourse.bass_utils import run_bass_kernel_spmd

N_CORES = 8
P = 128  # partitions / block size / dst window size
NWIN = 98  # windows per core
GW = 14  # windows per dedup group
NG = NWIN // GW  # 7 groups
GCH = 4096  # gather chunk: slots per dma_gather instruction


def _preprocess(n_nodes, edge_index, edge_weight):
    """Bin-pack dsts, build per-core tapes + dedup row pools."""
    nbins = N_CORES * NWIN
    dst = edge_index[0].astype(np.int64)
    src = edge_index[1].astype(np.int64)
    w = edge_weight.astype(np.float32)
    E = dst.shape[0]

    # --- LPT bin-packing of dsts into 784 windows (cap 128 dsts each) ---
    cnt_dst = np.bincount(dst, minlength=n_nodes)
    order = np.argsort(-cnt_dst, kind="stable")
    heap = [(0, b) for b in range(nbins)]  # (sum, bin)
    heapq.heapify(heap)
    bin_of_dst = np.empty(n_nodes, np.int64)
    off_of_dst = np.empty(n_nodes, np.int64)
    bin_fill = np.zeros(nbins, np.int64)
    stash = []
    for d in order:
        while True:
            s, b = heapq.heappop(heap)
            if bin_fill[b] < P:
                break
            stash.append(None)  # full bin: drop from heap
        bin_of_dst[d] = b
        off_of_dst[d] = bin_fill[b]
        bin_fill[b] += 1
        heapq.heappush(heap, (s + cnt_dst[d], b))
    core_of_bin = np.arange(nbins) // NWIN
    win_of_bin = np.arange(nbins) % NWIN

    core = core_of_bin[bin_of_dst[dst]]
    win = win_of_bin[bin_of_dst[dst]]
    off = off_of_dst[dst].astype(np.float32)

    # uniform blocks per window
    wcnt = np.bincount(bin_of_dst[dst], minlength=nbins)
    BW = int(-(-wcnt.max() // P))
    WSL = BW * P  # slots per window
    B = NWIN * BW  # blocks per core
    SL = B * P  # slots per core

    # --- per-(core,group) dedup of srcs; local idx for each edge ---
    group = win // GW
    cg = core * NG + group  # 0..55
    key = cg * n_nodes + src
    uniq, inv = np.unique(key, return_inverse=True)
    seg_of_uniq = uniq // n_nodes
    seg_sizes = np.bincount(seg_of_uniq, minlength=N_CORES * NG)
    U_max = int(seg_sizes.max())
    assert U_max <= 32767, f"group dedup overflow: {U_max}"
    seg_start = np.concatenate([[0], np.cumsum(seg_sizes)])[:-1]
    idx_local = (inv - seg_start[cg]).astype(np.int16)

    # --- slot position of each edge: window-major, arrival order ---
    wkey = (core * NWIN + win).astype(np.int64)
    order_e = np.argsort(wkey, kind="stable")
    swkey = wkey[order_e]
    starts = np.r_[0, np.flatnonzero(np.diff(swkey)) + 1]
    run_len = np.diff(np.r_[starts, E])
    run_id = np.repeat(np.arange(len(starts)), run_len)
    pos_in_win = np.arange(E) - starts[run_id]
    slot = (swkey % NWIN) * WSL + pos_in_win  # per-core slot

    idx_arr = np.zeros((N_CORES, SL), np.int16)
    off_arr = np.zeros((N_CORES, SL), np.float32)
    w_arr = np.zeros((N_CORES, SL), np.float32)
    flat = (swkey // NWIN) * SL + slot
    idx_arr.reshape(-1)[flat] = idx_local[order_e]
    off_arr.reshape(-1)[flat] = off[order_e]
    w_arr.reshape(-1)[flat] = w[order_e]

    # idx tape wrapped in 16 partitions, replicated 8x: idx[16g+p, s] = tape[16s+p]
    idxw = idx_arr.reshape(N_CORES, SL // 16, 16).transpose(0, 2, 1)
    idx_np = np.tile(idxw, (1, 8, 1)).copy()  # [C, 128, SL//16]

    # metadata columns: [128, B] bf16, column b = slots [b*128, (b+1)*128)
    dstf = (
        off_arr.reshape(N_CORES, B, P).transpose(0, 2, 1).astype(ml_dtypes.bfloat16)
    )
    wf = w_arr.reshape(N_CORES, B, P).transpose(0, 2, 1).astype(ml_dtypes.bfloat16)

    # per-core dedup row pools (filled later with x data)
    rows_of_uniq = uniq % n_nodes
    return dict(
        idx=idx_np,
        dstf=dstf,
        wf=wf,
        B=B,
        BW=BW,
        U_max=U_max,
        seg_sizes=seg_sizes,
        seg_start=seg_start,
        rows_of_uniq=rows_of_uniq,
        bin_of_dst=bin_of_dst,
        off_of_dst=off_of_dst,
    )


def _build_xg(x_bf, pp):
    """Per-core [NG*U_max, in_dim] bf16 dedup row pools."""
    n, in_dim = x_bf.shape
    U_max = pp["U_max"]
    xg = np.zeros((N_CORES, NG * U_max, in_dim), ml_dtypes.bfloat16)
    for c in range(N_CORES):
        for g in range(NG):
            s = c * NG + g
            rows = pp["rows_of_uniq"][
                pp["seg_start"][s] : pp["seg_start"][s] + pp["seg_sizes"][s]
            ]
            xg[c, g * U_max : g * U_max + len(rows)] = x_bf[rows]
    return xg


def _build_program(in_dim, out_dim, pp):
    B, BW, U_max = pp["B"], pp["BW"], pp["U_max"]
    SL = B * P

    nc = bacc.Bacc(
        "TRN2",
        target_bir_lowering=False,
        debug=False,
        num_devices=N_CORES,
        num_swdge_queues=4,
        dynamic_dma_scratch_size=65536,
    )
    f32 = mybir.dt.float32
    bf16 = mybir.dt.bfloat16
    i16 = mybir.dt.int16

    xg_d = nc.declare_dram_parameter("xg", [NG * U_max, in_dim], bf16, isOutput=False)
    idx_d = nc.declare_dram_parameter("idx", [P, SL // 16], i16, isOutput=False)
    dstf_d = nc.declare_dram_parameter("dstf", [P, B], bf16, isOutput=False)
    wf_d = nc.declare_dram_parameter("wf", [P, B], bf16, isOutput=False)
    iota_d = nc.declare_dram_parameter("iotab", [P, P], bf16, isOutput=False)
    wmat_d = nc.declare_dram_parameter("wmat", [in_dim, out_dim], f32, isOutput=False)
    bias_d = nc.declare_dram_parameter("biasrep", [P, out_dim], f32, isOutput=False)
    out_d = nc.declare_dram_parameter("out", [NWIN * P, out_dim], f32, isOutput=True)

    eq, mu = mybir.AluOpType.is_equal, mybir.AluOpType.mult
    # gather chunks: per group, GW*BW blocks chopped into GCH-slot chunks
    CB = GCH // P  # blocks per chunk
    chunks = []  # (group, block_start, n_blocks)
    for g in range(NG):
        b0 = g * GW * BW
        bend = (g + 1) * GW * BW
        while b0 < bend:
            nb = min(CB, bend - b0)
            chunks.append((g, b0, nb))
            b0 += nb

    GSL = GW * BW * P  # slots per group

    with tile.TileContext(nc) as tc:
        with (
            tc.tile_pool(name="const", bufs=1) as const_tp,
            tc.tile_pool(name="meta", bufs=1) as meta_tp,
            tc.tile_pool(name="g", bufs=5) as g_tp,
            tc.tile_pool(name="s", bufs=3) as s_tp,
            tc.tile_pool(name="t1", bufs=2) as t1_tp,
            tc.tile_pool(name="aggsb", bufs=3) as agg_tp,
            tc.tile_pool(name="outsb", bufs=3) as outsb_tp,
            tc.tile_pool(name="psum_agg", bufs=6, space="PSUM") as psum_agg_tp,
            tc.tile_pool(name="psum_out", bufs=2, space="PSUM") as psum_out_tp,
        ):
            wmat_t = const_tp.tile([in_dim, out_dim], f32)
            nc.sync.dma_start(out=wmat_t[:], in_=wmat_d[:, :])
            bias_t = const_tp.tile([P, out_dim], f32)
            nc.sync.dma_start(out=bias_t[:], in_=bias_d[:, :])
            iota_t = const_tp.tile([P, P], bf16)
            nc.sync.dma_start(out=iota_t[:], in_=iota_d[:, :])

            dstf_t = meta_tp.tile([P, B], bf16)
            nc.sync.dma_start(out=dstf_t[:], in_=dstf_d[:, :])
            wf_t = meta_tp.tile([P, B], bf16)
            nc.sync.dma_start(out=wf_t[:], in_=wf_d[:, :])
            # idx tape loaded per group so the first gather starts early
            idx_ts = []
            for g in range(NG):
                it = meta_tp.tile([P, GSL // 16], i16, tag=f"idx{g}")
                nc.sync.dma_start(
                    out=it[:],
                    in_=idx_d[:, g * GSL // 16 : (g + 1) * GSL // 16],
                )
                idx_ts.append(it)

            # per chunk: gather tile + S tile (built with 2 wide DVE ops)
            g_tiles = {}

            def ensure_chunk(ci):
                if ci in g_tiles:
                    return
                g, b0, nb = chunks[ci]
                g_t = g_tp.tile([P, nb * in_dim], bf16, tag="g")
                lb0 = b0 - g * GW * BW  # block offset within group
                nc.gpsimd.dma_gather(
                    out_ap=g_t[:].rearrange("p (c e) -> p c e", e=in_dim),
                    in_ap=xg_d[g * U_max :, :],
                    idxs_ap=idx_ts[g][:, lb0 * P // 16 : (lb0 + nb) * P // 16],
                    num_idxs=nb * P,
                    num_idxs_reg=nb * P,
                    elem_size=in_dim,
                    single_packet=False,
                    queue_num=ci % 4,
                )
                t1 = t1_tp.tile([P, nb * P], bf16, tag="t1")
                s_t = s_tp.tile([P, nb * P], bf16, tag="s")
                iota_v = (
                    iota_t[:]
                    .rearrange("p (u e) -> p u e", u=1)
                    .broadcast_to((P, nb, P))
                )
                dst_v = (
                    dstf_t[:, b0 : b0 + nb]
                    .rearrange("p (b u) -> p b u", u=1)
                    .broadcast_to((P, nb, P))
                )
                w_v = (
                    wf_t[:, b0 : b0 + nb]
                    .rearrange("p (b u) -> p b u", u=1)
                    .broadcast_to((P, nb, P))
                )
                t1_v = t1[:].rearrange("p (b e) -> p b e", e=P)
                s_v = s_t[:].rearrange("p (b e) -> p b e", e=P)
                nc.vector.tensor_tensor(out=t1_v, in0=iota_v, in1=dst_v, op=eq)
                nc.vector.tensor_tensor(out=s_v, in0=t1_v, in1=w_v, op=mu)
                g_tiles[ci] = (g_t, s_t, b0)

            for w in range(NWIN):
                psum_w = psum_agg_tp.tile([in_dim, P], f32, tag="aggT")
                for j in range(BW):
                    b = w * BW + j
                    ci = b // CB
                    ensure_chunk(ci)
                    g_t, s_t, b0 = g_tiles[ci]
                    rel = b - b0
                    nc.tensor.matmul(
                        out=psum_w[:],
                        lhsT=g_t[:, rel * in_dim : (rel + 1) * in_dim],
                        rhs=s_t[:, rel * P : (rel + 1) * P],
                        start=(j == 0),
                        stop=(j == BW - 1),
                    )
                agg_sb = agg_tp.tile([in_dim, P], f32, tag="aggsb")
                nc.scalar.copy(out=agg_sb[:], in_=psum_w[:])
                out_ps = psum_out_tp.tile([P, out_dim], f32, tag="outps")
                nc.tensor.matmul(
                    out=out_ps[:], lhsT=agg_sb[:], rhs=wmat_t[:], start=True, stop=True
                )
                out_sb = outsb_tp.tile([P, out_dim], f32, tag="outsb")
                nc.vector.tensor_add(out=out_sb[:], in0=out_ps[:], in1=bias_t[:])
                nc.scalar.dma_start(out=out_d[w * P : (w + 1) * P, :], in_=out_sb[:])

    nc.compile()
    return nc


def _prepare(x, edge_index, edge_weight, weight, bias):
    x = np.asarray(x, np.float32)
    edge_index = np.asarray(edge_index, np.int32)
    edge_weight = np.asarray(edge_weight, np.float32)
    weight = np.asarray(weight, np.float32)
    bias = np.asarray(bias, np.float32)

    n_nodes, in_dim = x.shape
    out_dim = weight.shape[1]

    pp = _preprocess(n_nodes, edge_index, edge_weight)
    nc = _build_program(in_dim, out_dim, pp)

    xg = _build_xg(x.astype(ml_dtypes.bfloat16), pp)
    iotab = np.broadcast_to(
        np.arange(P, dtype=np.float32), (P, P)
    ).astype(ml_dtypes.bfloat16)
    biasrep = np.broadcast_to(bias, (P, out_dim)).astype(np.float32).copy()
    in_maps = [
        {
            "xg": xg[c],
            "idx": pp["idx"][c],
            "dstf": pp["dstf"][c],
            "wf": pp["wf"][c],
            "iotab": iotab.copy(),
            "wmat": weight,
            "biasrep": biasrep,
        }
        for c in range(N_CORES)
    ]
    return nc, in_maps, pp, n_nodes, out_dim


def _collect(res, pp, n_nodes, out_dim):
    out = np.zeros((n_nodes, out_dim), np.float32)
    bin_of_dst, off_of_dst = pp["bin_of_dst"], pp["off_of_dst"]
    dsts = np.arange(n_nodes)
    c = bin_of_dst // NWIN
    row = (bin_of_dst % NWIN) * P + off_of_dst
    for ci in range(N_CORES):
        m = c == ci
        out[dsts[m]] = res.results[ci]["out"][row[m]]
    return out


def kernel(x, edge_index, edge_weight, weight, bias):
    nc, in_maps, pp, n_nodes, out_dim = _prepare(
        x, edge_index, edge_weight, weight, bias
    )
    res = run_bass_kernel_spmd(nc, in_maps, core_ids=list(range(N_CORES)))
    return _collect(res, pp, n_nodes, out_dim)


if __name__ == "__main__":
    rng = np.random.default_rng(0)
    N, E, DI, DO = 100000, 1600000, 128, 64
    if len(sys.argv) > 1 and sys.argv[1] == "small":
        N, E = 20000, 320000
    x = rng.standard_normal((N, DI), dtype=np.float32)
    ei = rng.integers(0, N, (2, E)).astype(np.int32)
    ew = rng.random(E, dtype=np.float32)
    wm = rng.standard_normal((DI, DO), dtype=np.float32) * 0.125
    bs = rng.standard_normal(DO, dtype=np.float32)

    out = kernel(x, ei, ew, wm, bs)

    h = x @ wm
    ref = np.zeros((N, DO), np.float32)
    np.add.at(ref, ei[0], ew[:, None] * h[ei[1]])
    ref += bs
    err = np.abs(out - ref).max() / (np.abs(ref).max() + 1e-9)
    print("max rel err:", err)


# revision 13
# speedup vs baseline: 3.2246x; 1.4894x over previous
"""GCNConv Trainium2 kernel: out = segment_sum(w_e * (x @ W)[src_e] -> dst_e) + bias.

Distribution (8-core SPMD, one program):
  - Destination nodes are bin-packed (LPT over per-dst edge counts) into
    8*98 = 784 windows of <=128 dsts each, so every (core, window) has an
    almost equal edge count; windows pad to a uniform 16 blocks of 128 edges.
  - Aggregation runs in x-space (in_dim features): per 128-edge block one PE
    matmul aggT += Xg^T @ S accumulates into the window's PSUM tile; at window
    end aggT moves to SBUF and out = aggT^T @ W + bias is stored.

Per core:
  - 98 windows are split into 7 groups of 14. For each (core, group) the
    host stores the deduplicated x rows used by that group's edges in a
    per-core DRAM pool (~25k rows < int16 gather reach), so each gather is
    one big 4096-row dma_gather at a single base.
  - S ([128 edges, 128 dst] scaled one-hot) is built on-device per 4096-slot
    gather chunk with two wide DVE tensor_tensor ops on stride-0 broadcast
    views: t1 = (iota == dstoff_bcast); S = t1 * w_bcast -- no S streaming
    from DRAM, no per-block scalar-pointer ops.
"""

import sys

sys.path.insert(0, "/opt/trn_rl_repo")

import heapq

import ml_dtypes
import numpy as np

from concourse import bacc, bass, mybir, tile
from concourse.bass_utils import run_bass_kernel_spmd

N_CORES = 8
P = 128  # partitions / block size / dst window size
NWIN = 98  # windows per core
GW = 14  # windows per dedup group
NG = NWIN // GW  # 7 groups
GCH = 4096  # gather chunk: slots per dma_gather instruction


def _preprocess(n_nodes, edge_index, edge_weight):
    """Bin-pack dsts, build per-core tapes + dedup row pools."""
    nbins = N_CORES * NWIN
    dst = edge_index[0].astype(np.int64)
    src = edge_index[1].astype(np.int64)
    w = edge_weight.astype(np.float32)
    E = dst.shape[0]

    # --- LPT bin-packing of dsts into 784 windows (cap 128 dsts each) ---
    cnt_dst = np.bincount(dst, minlength=n_nodes)
    order = np.argsort(-cnt_dst, kind="stable")
    heap = [(0, b) for b in range(nbins)]  # (sum, bin)
    heapq.heapify(heap)
    bin_of_dst = np.empty(n_nodes, np.int64)
    off_of_dst = np.empty(n_nodes, np.int64)
    bin_fill = np.zeros(nbins, np.int64)
    stash = []
    for d in order:
        while True:
            s, b = heapq.heappop(heap)
            if bin_fill[b] < P:
                break
            stash.append(None)  # full bin: drop from heap
        bin_of_dst[d] = b
        off_of_dst[d] = bin_fill[b]
        bin_fill[b] += 1
        heapq.heappush(heap, (s + cnt_dst[d], b))
    core_of_bin = np.arange(nbins) // NWIN
    win_of_bin = np.arange(nbins) % NWIN

    core = core_of_bin[bin_of_dst[dst]]
    win = win_of_bin[bin_of_dst[dst]]
    off = off_of_dst[dst].astype(np.float32)

    # uniform blocks per window
    wcnt = np.bincount(bin_of_dst[dst], minlength=nbins)
    BW = int(-(-wcnt.max() // P))
    WSL = BW * P  # slots per window
    B = NWIN * BW  # blocks per core
    SL = B * P  # slots per core

    # --- per-(core,group) dedup of srcs; local idx for each edge ---
    group = win // GW
    cg = core * NG + group  # 0..55
    key = cg * n_nodes + src
    uniq, inv = np.unique(key, return_inverse=True)
    seg_of_uniq = uniq // n_nodes
    seg_sizes = np.bincount(seg_of_uniq, minlength=N_CORES * NG)
    U_max = int(seg_sizes.max())
    assert U_max <= 32767, f"group dedup overflow: {U_max}"
    seg_start = np.concatenate([[0], np.cumsum(seg_sizes)])[:-1]
    idx_local = (inv - seg_start[cg]).astype(np.int16)

    # --- slot position of each edge: window-major, arrival order ---
    wkey = (core * NWIN + win).astype(np.int64)
    order_e = np.argsort(wkey, kind="stable")
    swkey = wkey[order_e]
    starts = np.r_[0, np.flatnonzero(np.diff(swkey)) + 1]
    run_len = np.diff(np.r_[starts, E])
    run_id = np.repeat(np.arange(len(starts)), run_len)
    pos_in_win = np.arange(E) - starts[run_id]
    slot = (swkey % NWIN) * WSL + pos_in_win  # per-core slot

    idx_arr = np.zeros((N_CORES, SL), np.int16)
    off_arr = np.zeros((N_CORES, SL), np.float32)
    w_arr = np.zeros((N_CORES, SL), np.float32)
    flat = (swkey // NWIN) * SL + slot
    idx_arr.reshape(-1)[flat] = idx_local[order_e]
    off_arr.reshape(-1)[flat] = off[order_e]
    w_arr.reshape(-1)[flat] = w[order_e]

    # idx tape wrapped in 16 partitions, replicated 8x: idx[16g+p, s] = tape[16s+p]
    idxw = idx_arr.reshape(N_CORES, SL // 16, 16).transpose(0, 2, 1)
    idx_np = np.tile(idxw, (1, 8, 1)).copy()  # [C, 128, SL//16]

    # precomputed S rows for streamed chunks: s_host[c, p, b*P + dstoff] = w
    s_host = np.zeros((N_CORES, P, SL), ml_dtypes.bfloat16)
    ci = np.arange(N_CORES)[:, None]
    bi = np.arange(B)[None, :]
    lane = np.arange(P)
    colbase = bi * P
    offs = off_arr.reshape(N_CORES, B, P).astype(np.int64)
    vals = w_arr.reshape(N_CORES, B, P).astype(ml_dtypes.bfloat16)
    s_host[
        ci[:, :, None],
        lane[None, None, :],
        colbase[:, :, None] + offs,
    ] = vals

    # metadata columns: [128, B] bf16, column b = slots [b*128, (b+1)*128)
    dstf = (
        off_arr.reshape(N_CORES, B, P).transpose(0, 2, 1).astype(ml_dtypes.bfloat16)
    )
    wf = w_arr.reshape(N_CORES, B, P).transpose(0, 2, 1).astype(ml_dtypes.bfloat16)

    # per-core dedup row pools (filled later with x data)
    rows_of_uniq = uniq % n_nodes
    return dict(
        idx=idx_np,
        dstf=dstf,
        wf=wf,
        s_host=s_host,
        B=B,
        BW=BW,
        U_max=U_max,
        seg_sizes=seg_sizes,
        seg_start=seg_start,
        rows_of_uniq=rows_of_uniq,
        bin_of_dst=bin_of_dst,
        off_of_dst=off_of_dst,
    )


def _build_xg(x_bf, pp):
    """Per-core [NG*U_max, in_dim] bf16 dedup row pools."""
    n, in_dim = x_bf.shape
    U_max = pp["U_max"]
    xg = np.zeros((N_CORES, NG * U_max, in_dim), ml_dtypes.bfloat16)
    for c in range(N_CORES):
        for g in range(NG):
            s = c * NG + g
            rows = pp["rows_of_uniq"][
                pp["seg_start"][s] : pp["seg_start"][s] + pp["seg_sizes"][s]
            ]
            xg[c, g * U_max : g * U_max + len(rows)] = x_bf[rows]
    return xg


def _build_program(in_dim, out_dim, pp):
    B, BW, U_max = pp["B"], pp["BW"], pp["U_max"]
    SL = B * P

    nc = bacc.Bacc(
        "TRN2",
        target_bir_lowering=False,
        debug=False,
        num_devices=N_CORES,
        num_swdge_queues=4,
        dynamic_dma_scratch_size=65536,
    )
    f32 = mybir.dt.float32
    bf16 = mybir.dt.bfloat16
    i16 = mybir.dt.int16

    xg_d = nc.declare_dram_parameter("xg", [NG * U_max, in_dim], bf16, isOutput=False)
    idx_d = nc.declare_dram_parameter("idx", [P, SL // 16], i16, isOutput=False)
    dstf_d = nc.declare_dram_parameter("dstf", [P, B], bf16, isOutput=False)
    wf_d = nc.declare_dram_parameter("wf", [P, B], bf16, isOutput=False)
    smat_d = nc.declare_dram_parameter("smat", [P, SL], bf16, isOutput=False)
    iota_d = nc.declare_dram_parameter("iotab", [P, P], bf16, isOutput=False)
    wmat_d = nc.declare_dram_parameter("wmat", [in_dim, out_dim], f32, isOutput=False)
    bias_d = nc.declare_dram_parameter("biasrep", [P, out_dim], f32, isOutput=False)
    out_d = nc.declare_dram_parameter("out", [NWIN * P, out_dim], f32, isOutput=True)

    eq, mu = mybir.AluOpType.is_equal, mybir.AluOpType.mult
    # gather chunks: split each group into equal chunks of <= GCH slots
    gblk = GW * BW
    nch = -(-gblk * P // GCH)
    assert gblk % nch == 0, (gblk, nch)
    CB = gblk // nch  # blocks per chunk (uniform)
    chunks = []  # (group, block_start, n_blocks)
    for g in range(NG):
        for k in range(nch):
            chunks.append((g, g * gblk + k * CB, CB))

    GSL = GW * BW * P  # slots per group

    with tile.TileContext(nc) as tc:
        with (
            tc.tile_pool(name="const", bufs=1) as const_tp,
            tc.tile_pool(name="meta", bufs=1) as meta_tp,
            tc.tile_pool(name="g", bufs=5) as g_tp,
            tc.tile_pool(name="s", bufs=3) as s_tp,
            tc.tile_pool(name="t1", bufs=2) as t1_tp,
            tc.tile_pool(name="aggsb", bufs=3) as agg_tp,
            tc.tile_pool(name="outsb", bufs=3) as outsb_tp,
            tc.tile_pool(name="psum_agg", bufs=6, space="PSUM") as psum_agg_tp,
            tc.tile_pool(name="psum_out", bufs=2, space="PSUM") as psum_out_tp,
        ):
            wmat_t = const_tp.tile([in_dim, out_dim], f32)
            nc.sync.dma_start(out=wmat_t[:], in_=wmat_d[:, :])
            bias_t = const_tp.tile([P, out_dim], f32)
            nc.sync.dma_start(out=bias_t[:], in_=bias_d[:, :])
            iota_t = const_tp.tile([P, P], bf16)
            nc.sync.dma_start(out=iota_t[:], in_=iota_d[:, :])

            dstf_t = meta_tp.tile([P, B], bf16)
            nc.sync.dma_start(out=dstf_t[:], in_=dstf_d[:, :])
            wf_t = meta_tp.tile([P, B], bf16)
            nc.sync.dma_start(out=wf_t[:], in_=wf_d[:, :])
            # idx tape loaded per group so the first gather starts early
            idx_ts = []
            for g in range(NG):
                it = meta_tp.tile([P, GSL // 16], i16, tag=f"idx{g}")
                nc.sync.dma_start(
                    out=it[:],
                    in_=idx_d[:, g * GSL // 16 : (g + 1) * GSL // 16],
                )
                idx_ts.append(it)

            # per chunk: gather tile + S tile (built with 2 wide DVE ops)
            g_tiles = {}

            def ensure_chunk(ci):
                if ci in g_tiles:
                    return
                g, b0, nb = chunks[ci]
                g_t = g_tp.tile([P, nb * in_dim], bf16, tag="g")
                lb0 = b0 - g * GW * BW  # block offset within group
                nc.gpsimd.dma_gather(
                    out_ap=g_t[:].rearrange("p (c e) -> p c e", e=in_dim),
                    in_ap=xg_d[g * U_max :, :],
                    idxs_ap=idx_ts[g][:, lb0 * P // 16 : (lb0 + nb) * P // 16],
                    num_idxs=nb * P,
                    num_idxs_reg=nb * P,
                    elem_size=in_dim,
                    single_packet=False,
                    queue_num=ci % 4,
                )
                s_t = s_tp.tile([P, nb * P], bf16, tag="s")
                if ci % 10 < 7:
                    # streamed: S rows precomputed host-side
                    nc.scalar.dma_start(
                        out=s_t[:], in_=smat_d[:, b0 * P : (b0 + nb) * P]
                    )
                else:
                    # built on-device: (iota == dstoff_bcast) * w_bcast
                    t1 = t1_tp.tile([P, nb * P], bf16, tag="t1")
                    iota_v = (
                        iota_t[:]
                        .rearrange("p (u e) -> p u e", u=1)
                        .broadcast_to((P, nb, P))
                    )
                    dst_v = (
                        dstf_t[:, b0 : b0 + nb]
                        .rearrange("p (b u) -> p b u", u=1)
                        .broadcast_to((P, nb, P))
                    )
                    w_v = (
                        wf_t[:, b0 : b0 + nb]
                        .rearrange("p (b u) -> p b u", u=1)
                        .broadcast_to((P, nb, P))
                    )
                    t1_v = t1[:].rearrange("p (b e) -> p b e", e=P)
                    s_v = s_t[:].rearrange("p (b e) -> p b e", e=P)
                    nc.vector.tensor_tensor(out=t1_v, in0=iota_v, in1=dst_v, op=eq)
                    nc.vector.tensor_tensor(out=s_v, in0=t1_v, in1=w_v, op=mu)
                g_tiles[ci] = (g_t, s_t, b0)

            for w in range(NWIN):
                psum_w = psum_agg_tp.tile([in_dim, P], f32, tag="aggT")
                for j in range(BW):
                    b = w * BW + j
                    ci = b // CB
                    ensure_chunk(ci)
                    g_t, s_t, b0 = g_tiles[ci]
                    rel = b - b0
                    nc.tensor.matmul(
                        out=psum_w[:],
                        lhsT=g_t[:, rel * in_dim : (rel + 1) * in_dim],
                        rhs=s_t[:, rel * P : (rel + 1) * P],
                        start=(j == 0),
                        stop=(j == BW - 1),
                    )
                agg_sb = agg_tp.tile([in_dim, P], f32, tag="aggsb")
                nc.scalar.copy(out=agg_sb[:], in_=psum_w[:])
                out_ps = psum_out_tp.tile([P, out_dim], f32, tag="outps")
                nc.tensor.matmul(
                    out=out_ps[:], lhsT=agg_sb[:], rhs=wmat_t[:], start=True, stop=True
                )
                out_sb = outsb_tp.tile([P, out_dim], f32, tag="outsb")
                nc.vector.tensor_add(out=out_sb[:], in0=out_ps[:], in1=bias_t[:])
                nc.scalar.dma_start(out=out_d[w * P : (w + 1) * P, :], in_=out_sb[:])

    nc.compile()
    return nc


def _prepare(x, edge_index, edge_weight, weight, bias):
    x = np.asarray(x, np.float32)
    edge_index = np.asarray(edge_index, np.int32)
    edge_weight = np.asarray(edge_weight, np.float32)
    weight = np.asarray(weight, np.float32)
    bias = np.asarray(bias, np.float32)

    n_nodes, in_dim = x.shape
    out_dim = weight.shape[1]

    pp = _preprocess(n_nodes, edge_index, edge_weight)
    nc = _build_program(in_dim, out_dim, pp)

    xg = _build_xg(x.astype(ml_dtypes.bfloat16), pp)
    iotab = np.broadcast_to(
        np.arange(P, dtype=np.float32), (P, P)
    ).astype(ml_dtypes.bfloat16)
    biasrep = np.broadcast_to(bias, (P, out_dim)).astype(np.float32).copy()
    in_maps = [
        {
            "xg": xg[c],
            "idx": pp["idx"][c],
            "dstf": pp["dstf"][c],
            "wf": pp["wf"][c],
            "smat": pp["s_host"][c],
            "iotab": iotab.copy(),
            "wmat": weight,
            "biasrep": biasrep,
        }
        for c in range(N_CORES)
    ]
    return nc, in_maps, pp, n_nodes, out_dim


def _collect(res, pp, n_nodes, out_dim):
    out = np.zeros((n_nodes, out_dim), np.float32)
    bin_of_dst, off_of_dst = pp["bin_of_dst"], pp["off_of_dst"]
    dsts = np.arange(n_nodes)
    c = bin_of_dst // NWIN
    row = (bin_of_dst % NWIN) * P + off_of_dst
    for ci in range(N_CORES):
        m = c == ci
        out[dsts[m]] = res.results[ci]["out"][row[m]]
    return out


def kernel(x, edge_index, edge_weight, weight, bias):
    nc, in_maps, pp, n_nodes, out_dim = _prepare(
        x, edge_index, edge_weight, weight, bias
    )
    res = run_bass_kernel_spmd(nc, in_maps, core_ids=list(range(N_CORES)))
    return _collect(res, pp, n_nodes, out_dim)


if __name__ == "__main__":
    rng = np.random.default_rng(0)
    N, E, DI, DO = 100000, 1600000, 128, 64
    if len(sys.argv) > 1 and sys.argv[1] == "small":
        N, E = 20000, 320000
    x = rng.standard_normal((N, DI), dtype=np.float32)
    ei = rng.integers(0, N, (2, E)).astype(np.int32)
    ew = rng.random(E, dtype=np.float32)
    wm = rng.standard_normal((DI, DO), dtype=np.float32) * 0.125
    bs = rng.standard_normal(DO, dtype=np.float32)

    out = kernel(x, ei, ew, wm, bs)

    h = x @ wm
    ref = np.zeros((N, DO), np.float32)
    np.add.at(ref, ei[0], ew[:, None] * h[ei[1]])
    ref += bs
    err = np.abs(out - ref).max() / (np.abs(ref).max() + 1e-9)
    print("max rel err:", err)
